# revision 6
# baseline (speedup 1.0000x reference)
"""Trainium2 Bass kernel for the Lure-system rollout.

Math (per batch column, all fp32):
    w_t = tanh(C2 x_t + D21 d_t)
    e_t = C x_t + D d_t + D12 w_t
    x_{t+1} = A x_t + B d_t + B2 w_t

Device formulation (per core, batch shard of 32 columns):
    state tile s_t = [w_t ; x_t]  (128 partitions x 32 batch)
    psum tile P_{t+1} = [z_{t+1} ; x_{t+1}]:
        P_{t+1}  = Wd^T @ [d_t ; d_{t+1}]          (start)
        P_{t+1} += Ws^T @ s_t                      (stop)
      where Ws packs [G0=C2B2, C2A | B2, A] and Wd packs [C2B, D21 | B, 0].
    w_{t+1} = tanh(P_{t+1}[0:64])  (ACT, PSUM->SBUF)
    x_{t+1} = copy P_{t+1}[64:128] (DVE, PSUM->SBUF)
    e computed in bulk every 16 steps from the [w;x] history (off critical path).

The serial critical path is exactly: tanh -> Ws-matmul -> tanh (2 sem hops/step).
"""

import numpy as np

import concourse.bacc as bacc
import concourse.bass as bass
import concourse.mybir as mybir
import concourse.tile as tile
from concourse.bass_utils import run_bass_kernel_spmd

NX, ND, NE, NW = 64, 16, 16, 64
BFULL, NSTEPS = 256, 1024
NCORES = 8
BC = BFULL // NCORES  # batch per core

F32 = mybir.dt.float32


def _host_weights(A, B, C, D, B2, C2, D12, D21):
    A, B, C, D = (np.asarray(m, np.float64) for m in (A, B, C, D))
    B2, C2, D12, D21 = (np.asarray(m, np.float64) for m in (B2, C2, D12, D21))
    G0 = C2 @ B2          # [64, 64] w -> z
    C2A = C2 @ A          # [64, 64] x -> z
    C2B = C2 @ B          # [64, 16] d_t -> z

    Ws = np.zeros((128, 128), np.float64)
    Ws[0:64, 0:64] = G0.T
    Ws[64:128, 0:64] = C2A.T
    Ws[0:64, 64:128] = B2.T
    Ws[64:128, 64:128] = A.T

    Wd = np.zeros((32, 128), np.float64)
    Wd[0:16, 0:64] = C2B.T
    Wd[16:32, 0:64] = D21.T
    Wd[0:16, 64:128] = B.T

    Wdm1 = np.zeros((32, 64), np.float64)
    Wdm1[0:16, :] = D21.T

    W0 = C2.T  # [64, 64], used at partition base 64 for z_0 += C2 x_0

    We = np.zeros((128, 16), np.float64)
    We[0:64, :] = D12.T
    We[64:128, :] = C.T

    WeD = D.T  # [16, 16]

    f32 = np.float32
    return {
        "Ws": np.ascontiguousarray(Ws, f32),
        "Wd": np.ascontiguousarray(Wd, f32),
        "Wdm1": np.ascontiguousarray(Wdm1, f32),
        "W0": np.ascontiguousarray(W0, f32),
        "We": np.ascontiguousarray(We, f32),
        "WeD": np.ascontiguousarray(WeD, f32),
    }


def _build_bass(nsteps, bc, ch=256, dma_chunk=64, e_chunk=16):
    """Emit the Bass/Tile program. Same NEFF on all cores (pure data parallel)."""
    nc = bacc.Bacc("TRN2", target_bir_lowering=False, debug=False)

    d_dup = nc.dram_tensor("d_dup", [32, nsteps, bc], F32, kind="ExternalInput").ap()
    x0 = nc.dram_tensor("x0", [64, bc], F32, kind="ExternalInput").ap()
    Ws = nc.dram_tensor("Ws", [128, 128], F32, kind="ExternalInput").ap()
    Wd = nc.dram_tensor("Wd", [32, 128], F32, kind="ExternalInput").ap()
    Wdm1 = nc.dram_tensor("Wdm1", [32, 64], F32, kind="ExternalInput").ap()
    W0 = nc.dram_tensor("W0", [64, 64], F32, kind="ExternalInput").ap()
    We = nc.dram_tensor("We", [128, 16], F32, kind="ExternalInput").ap()
    WeD = nc.dram_tensor("WeD", [16, 16], F32, kind="ExternalInput").ap()

    sx_out = nc.dram_tensor("sx_out", [128, nsteps + 1, bc], F32,
                            kind="ExternalOutput").ap()
    e_out = nc.dram_tensor("e_out", [16, nsteps, bc], F32,
                           kind="ExternalOutput").ap()

    assert ch % dma_chunk == 0 and ch % e_chunk == 0
    assert nsteps % dma_chunk == 0

    tanh = mybir.ActivationFunctionType.Tanh

    with tile.TileContext(nc) as tc:
        with (
            tc.tile_pool(name="const", bufs=1) as const,
            tc.tile_pool(name="hist", bufs=1) as hist,
            tc.tile_pool(name="pp", bufs=4, space="PSUM") as pp,
            tc.tile_pool(name="pe", bufs=2, space="PSUM") as pe,
        ):
            ws_sb = const.tile([128, 128], F32)
            wd_sb = const.tile([32, 128], F32)
            wdm1_sb = const.tile([32, 64], F32)
            w0_sb = const.tile([128, 64], F32)   # C2^T lives at partitions 64:128
            we_sb = const.tile([128, 16], F32)
            wed_sb = const.tile([16, 16], F32)
            nc.sync.dma_start(out=ws_sb, in_=Ws)
            nc.sync.dma_start(out=wd_sb, in_=Wd)
            nc.sync.dma_start(out=wdm1_sb, in_=Wdm1)
            nc.sync.dma_start(out=w0_sb[64:128, :], in_=W0)
            nc.sync.dma_start(out=we_sb, in_=We)
            nc.sync.dma_start(out=wed_sb, in_=WeD)

            s_hist = hist.tile([128, ch, bc], F32)   # [w ; x] per slot
            d_sb = hist.tile([32, ch, bc], F32)      # [d_t ; d_{t+1}] per slot
            e_stage = hist.tile([16, ch, bc], F32)

            nc.sync.dma_start(out=s_hist[64:128, 0, :], in_=x0)

            # d chunks 0 and 1 up front
            pre = min(2 * dma_chunk, nsteps)
            nc.sync.dma_start(out=d_sb[:, 0:pre, :], in_=d_dup[:, 0:pre, :])

            # P_0 = [z_0] = D21 d_0 + C2 x_0
            p_cur = pp.tile([64, bc], F32, tag="p0", bufs=1)
            nc.tensor.matmul(p_cur, wdm1_sb, d_sb[:, 0, :], start=True, stop=False)
            nc.tensor.matmul(p_cur, w0_sb[64:128, :], s_hist[64:128, 0, :],
                             start=False, stop=True)

            for t in range(nsteps):
                slot = t % ch

                # prefetch d chunk t//dma_chunk + 2
                if t % dma_chunk == 0:
                    lo = (t // dma_chunk + 2) * dma_chunk
                    if lo < nsteps:
                        hi = min(lo + dma_chunk, nsteps)
                        sl = lo % ch
                        nc.sync.dma_start(out=d_sb[:, sl:sl + (hi - lo), :],
                                          in_=d_dup[:, lo:hi, :])

                # w_t = tanh(z_t)
                nc.scalar.activation(out=s_hist[0:64, slot, :],
                                     in_=p_cur[0:64, :], func=tanh)

                # P_{t+1} = Wd^T [d_t; d_{t+1}] + Ws^T [w_t; x_t]
                p_next = pp.tile([128, bc], F32, tag="p")
                nc.tensor.matmul(p_next, wd_sb, d_sb[:, slot, :],
                                 start=True, stop=False)
                nc.tensor.matmul(p_next, ws_sb, s_hist[:, slot, :],
                                 start=False, stop=True)
                nc.vector.tensor_copy(s_hist[64:128, (t + 1) % ch, :],
                                      p_next[64:128, :])
                p_cur = p_next

                # e for steps [t-e_chunk+1, t]
                if t % e_chunk == e_chunk - 1:
                    sl0 = (t - e_chunk + 1) % ch
                    ep = pe.tile([16, e_chunk * bc], F32, tag="e")
                    nc.tensor.matmul(ep, we_sb, s_hist[:, sl0:sl0 + e_chunk, :],
                                     start=True, stop=False)
                    nc.tensor.matmul(ep, wed_sb, d_sb[0:16, sl0:sl0 + e_chunk, :],
                                     start=False, stop=True)
                    nc.vector.tensor_copy(e_stage[:, sl0:sl0 + e_chunk, :], ep)

                # stores
                if t % dma_chunk == dma_chunk - 1:
                    lo = t - dma_chunk + 1
                    sl = lo % ch
                    nc.sync.dma_start(out=sx_out[:, lo:t + 1, :],
                                      in_=s_hist[:, sl:sl + dma_chunk, :])
                e_flush = min(2 * dma_chunk, nsteps)
                if t % e_flush == e_flush - 1:
                    lo = t - e_flush + 1
                    sl = lo % ch
                    nc.sync.dma_start(out=e_out[:, lo:t + 1, :],
                                      in_=e_stage[:, sl:sl + e_flush, :])

            # final x_{nsteps} (written by step nsteps-1's copy)
            sl = nsteps % ch
            nc.sync.dma_start(out=sx_out[64:128, nsteps:nsteps + 1, :],
                              in_=s_hist[64:128, sl:sl + 1, :])

    nc.compile()
    return nc


def _make_d_dup(d_shard, nsteps, bc):
    # d_shard: [bc, nsteps, ND] -> [32, nsteps, bc] with rows 0:16 = d_t,
    # rows 16:32 = d_{t+1} (zeros at t = nsteps-1).
    dT = np.ascontiguousarray(np.transpose(d_shard, (2, 1, 0)), np.float32)
    d_dup = np.zeros((32, nsteps, bc), np.float32)
    d_dup[0:16] = dT
    d_dup[16:32, :-1] = dT[:, 1:]
    return d_dup


def run_device(d, x0, A, B, C, D, B2, C2, D12, D21, *, trace=False,
               nsteps=NSTEPS, ncores=NCORES, trace_kwargs=None):
    d = np.asarray(d, np.float32)
    x0 = np.asarray(x0, np.float32)
    bfull = d.shape[0]
    bc = bfull // ncores

    weights = _host_weights(A, B, C, D, B2, C2, D12, D21)
    nc = _build_bass(nsteps, bc)

    in_maps = []
    for c in range(ncores):
        ds = d[c * bc:(c + 1) * bc, :nsteps, :, 0]        # [bc, nsteps, ND]
        x0s = x0[c * bc:(c + 1) * bc, :, 0]               # [bc, NX]
        in_maps.append({
            "d_dup": _make_d_dup(ds, nsteps, bc),
            "x0": np.ascontiguousarray(x0s.T, np.float32),  # [64, bc]
            **weights,
        })

    res = run_bass_kernel_spmd(nc, in_maps, core_ids=list(range(ncores)),
                               trace=trace, **(trace_kwargs or {}))

    e_hat = np.empty((bfull, nsteps, NE, 1), np.float32)
    w = np.empty((bfull, nsteps, NW, 1), np.float32)
    x = np.empty((bfull, nsteps + 1, NX, 1), np.float32)
    for c in range(ncores):
        sx = res.results[c]["sx_out"]                     # [128, nsteps+1, bc]
        ec = res.results[c]["e_out"]                      # [16, nsteps, bc]
        sl = slice(c * bc, (c + 1) * bc)
        w[sl, :, :, 0] = np.transpose(sx[0:64, 0:nsteps, :], (2, 1, 0))
        x[sl, 1:, :, 0] = np.transpose(sx[64:128, 1:nsteps + 1, :], (2, 1, 0))
        e_hat[sl, :, :, 0] = np.transpose(ec, (2, 1, 0))
    x[:, 0, :, 0] = x0[:, :, 0]
    return (e_hat, x, w), res


def kernel(d, x0, A, B, C, D, B2, C2, D12, D21):
    (e_hat, x, w), _ = run_device(d, x0, A, B, C, D, B2, C2, D12, D21)
    d = np.asarray(d, np.float32)
    return (e_hat, (x, w), d)


# revision 10
# speedup vs baseline: 1.5564x; 1.5564x over previous
"""Trainium2 Bass kernel for the Lure-system rollout.

Math (per batch column, all fp32):
    w_t = tanh(C2 x_t + D21 d_t)
    e_t = C x_t + D d_t + D12 w_t
    x_{t+1} = A x_t + B d_t + B2 w_t

Device formulation (per core, batch shard of 32 columns):
    state tile s_t = [w_t ; x_t]  (128 partitions x 32 batch)
    psum tile P_{t+1} = [z_{t+1} ; x_{t+1}]:
        P_{t+1}  = Wd^T @ [d_t ; d_{t+1}]          (start)
        P_{t+1} += Ws^T @ s_t                      (stop)
      where Ws packs [G0=C2B2, C2A | B2, A] and Wd packs [C2B, D21 | B, 0].
    w_{t+1} = tanh(P_{t+1}[0:64])  (ACT, PSUM->SBUF)
    x_{t+1} = copy P_{t+1}[64:128] (DVE, PSUM->SBUF)
    e computed in bulk every 16 steps from the [w;x] history (off critical path).

The serial critical path is exactly: tanh -> Ws-matmul -> tanh (2 sem hops/step).
"""

import numpy as np

import concourse.bacc as bacc
import concourse.bass as bass
import concourse.mybir as mybir
import concourse.tile as tile
from concourse.bass_utils import run_bass_kernel_spmd

NX, ND, NE, NW = 64, 16, 16, 64
BFULL, NSTEPS = 256, 1024
NCORES = 8
BC = BFULL // NCORES  # batch per core

F32 = mybir.dt.float32


def _host_weights(A, B, C, D, B2, C2, D12, D21):
    A, B, C, D = (np.asarray(m, np.float64) for m in (A, B, C, D))
    B2, C2, D12, D21 = (np.asarray(m, np.float64) for m in (B2, C2, D12, D21))
    G0 = C2 @ B2          # [64, 64] w -> z
    C2A = C2 @ A          # [64, 64] x -> z
    C2B = C2 @ B          # [64, 16] d_t -> z

    Ws = np.zeros((128, 128), np.float64)
    Ws[0:64, 0:64] = G0.T
    Ws[64:128, 0:64] = C2A.T
    Ws[0:64, 64:128] = B2.T
    Ws[64:128, 64:128] = A.T

    Wd = np.zeros((32, 128), np.float64)
    Wd[0:16, 0:64] = C2B.T
    Wd[16:32, 0:64] = D21.T
    Wd[0:16, 64:128] = B.T

    Wdm1 = np.zeros((32, 64), np.float64)
    Wdm1[0:16, :] = D21.T

    W0 = C2.T  # [64, 64], used at partition base 64 for z_0 += C2 x_0

    We = np.zeros((128, 16), np.float64)
    We[0:64, :] = D12.T
    We[64:128, :] = C.T

    WeD = D.T  # [16, 16]

    f32 = np.float32
    return {
        "Ws": np.ascontiguousarray(Ws, f32),
        "Wd": np.ascontiguousarray(Wd, f32),
        "Wdm1": np.ascontiguousarray(Wdm1, f32),
        "W0": np.ascontiguousarray(W0, f32),
        "We": np.ascontiguousarray(We, f32),
        "WeD": np.ascontiguousarray(WeD, f32),
    }


def _build_bass(nsteps, bc, ch=256, dma_chunk=64, e_chunk=16, mm_dt=F32):
    """Emit the Bass/Tile program. Same NEFF on all cores (pure data parallel).

    mm_dt controls the dtype of the recurrence matmuls (weights, state
    history, d staging). PSUM stays fp32 throughout.
    """
    nc = bacc.Bacc("TRN2", target_bir_lowering=False, debug=False)

    DT = mm_dt
    d_dup = nc.dram_tensor("d_dup", [32, nsteps, bc], DT, kind="ExternalInput").ap()
    x0 = nc.dram_tensor("x0", [64, bc], DT, kind="ExternalInput").ap()
    Ws = nc.dram_tensor("Ws", [128, 128], DT, kind="ExternalInput").ap()
    Wd = nc.dram_tensor("Wd", [32, 128], DT, kind="ExternalInput").ap()
    Wdm1 = nc.dram_tensor("Wdm1", [32, 64], DT, kind="ExternalInput").ap()
    W0 = nc.dram_tensor("W0", [64, 64], DT, kind="ExternalInput").ap()
    We = nc.dram_tensor("We", [128, 16], DT, kind="ExternalInput").ap()
    WeD = nc.dram_tensor("WeD", [16, 16], DT, kind="ExternalInput").ap()

    sx_out = nc.dram_tensor("sx_out", [128, nsteps + 1, bc], DT,
                            kind="ExternalOutput").ap()
    e_out = nc.dram_tensor("e_out", [16, nsteps, bc], F32,
                           kind="ExternalOutput").ap()

    assert ch % dma_chunk == 0 and ch % e_chunk == 0
    assert nsteps % dma_chunk == 0

    tanh = mybir.ActivationFunctionType.Tanh

    with tile.TileContext(nc) as tc:
        with (
            tc.tile_pool(name="const", bufs=1) as const,
            tc.tile_pool(name="hist", bufs=1) as hist,
            tc.tile_pool(name="pp", bufs=4, space="PSUM") as pp,
            tc.tile_pool(name="pe", bufs=2, space="PSUM") as pe,
        ):
            ws_sb = const.tile([128, 128], DT)
            wd_sb = const.tile([32, 128], DT)
            wdm1_sb = const.tile([32, 64], DT)
            w0_sb = const.tile([128, 64], DT)   # C2^T lives at partitions 64:128
            we_sb = const.tile([128, 16], DT)
            wed_sb = const.tile([16, 16], DT)
            nc.sync.dma_start(out=ws_sb, in_=Ws)
            nc.sync.dma_start(out=wd_sb, in_=Wd)
            nc.sync.dma_start(out=wdm1_sb, in_=Wdm1)
            nc.sync.dma_start(out=w0_sb[64:128, :], in_=W0)
            nc.sync.dma_start(out=we_sb, in_=We)
            nc.sync.dma_start(out=wed_sb, in_=WeD)

            s_hist = hist.tile([128, ch, bc], DT)   # [w ; x] per slot
            d_sb = hist.tile([32, ch, bc], DT)      # [d_t ; d_{t+1}] per slot
            e_stage = hist.tile([16, ch, bc], F32)

            nc.sync.dma_start(out=s_hist[64:128, 0, :], in_=x0)

            # d chunks 0 and 1 up front
            pre = min(2 * dma_chunk, nsteps)
            nc.sync.dma_start(out=d_sb[:, 0:pre, :], in_=d_dup[:, 0:pre, :])

            # P_0 = [z_0] = D21 d_0 + C2 x_0
            p_cur = pp.tile([64, bc], F32, tag="p0", bufs=1)
            nc.tensor.matmul(p_cur, wdm1_sb, d_sb[:, 0, :], start=True, stop=False)
            nc.tensor.matmul(p_cur, w0_sb[64:128, :], s_hist[64:128, 0, :],
                             start=False, stop=True)

            for t in range(nsteps):
                slot = t % ch

                # prefetch d chunk t//dma_chunk + 2
                if t % dma_chunk == 0:
                    lo = (t // dma_chunk + 2) * dma_chunk
                    if lo < nsteps:
                        hi = min(lo + dma_chunk, nsteps)
                        sl = lo % ch
                        nc.sync.dma_start(out=d_sb[:, sl:sl + (hi - lo), :],
                                          in_=d_dup[:, lo:hi, :])

                # w_t = tanh(z_t)
                nc.scalar.activation(out=s_hist[0:64, slot, :],
                                     in_=p_cur[0:64, :], func=tanh)

                # P_{t+1} = Wd^T [d_t; d_{t+1}] + Ws^T [w_t; x_t]
                p_next = pp.tile([128, bc], F32, tag="p")
                nc.tensor.matmul(p_next, wd_sb, d_sb[:, slot, :],
                                 start=True, stop=False)
                nc.tensor.matmul(p_next, ws_sb, s_hist[:, slot, :],
                                 start=False, stop=True)
                nc.vector.tensor_copy(s_hist[64:128, (t + 1) % ch, :],
                                      p_next[64:128, :])
                p_cur = p_next

                # e for steps [t-e_chunk+1, t]
                if t % e_chunk == e_chunk - 1:
                    sl0 = (t - e_chunk + 1) % ch
                    ep = pe.tile([16, e_chunk * bc], F32, tag="e")
                    nc.tensor.matmul(ep, we_sb, s_hist[:, sl0:sl0 + e_chunk, :],
                                     start=True, stop=False)
                    nc.tensor.matmul(ep, wed_sb, d_sb[0:16, sl0:sl0 + e_chunk, :],
                                     start=False, stop=True)
                    nc.vector.tensor_copy(e_stage[:, sl0:sl0 + e_chunk, :], ep)

                # stores
                if t % dma_chunk == dma_chunk - 1:
                    lo = t - dma_chunk + 1
                    sl = lo % ch
                    nc.sync.dma_start(out=sx_out[:, lo:t + 1, :],
                                      in_=s_hist[:, sl:sl + dma_chunk, :])
                e_flush = min(2 * dma_chunk, nsteps)
                if t % e_flush == e_flush - 1:
                    lo = t - e_flush + 1
                    sl = lo % ch
                    nc.sync.dma_start(out=e_out[:, lo:t + 1, :],
                                      in_=e_stage[:, sl:sl + e_flush, :])

            # final x_{nsteps} (written by step nsteps-1's copy)
            sl = nsteps % ch
            nc.sync.dma_start(out=sx_out[64:128, nsteps:nsteps + 1, :],
                              in_=s_hist[64:128, sl:sl + 1, :])

    nc.compile()
    return nc


def _make_d_dup(d_shard, nsteps, bc):
    # d_shard: [bc, nsteps, ND] -> [32, nsteps, bc] with rows 0:16 = d_t,
    # rows 16:32 = d_{t+1} (zeros at t = nsteps-1).
    dT = np.ascontiguousarray(np.transpose(d_shard, (2, 1, 0)), np.float32)
    d_dup = np.zeros((32, nsteps, bc), np.float32)
    d_dup[0:16] = dT
    d_dup[16:32, :-1] = dT[:, 1:]
    return d_dup


MM_DT = "f32r"  # "f32" | "f32r" | "f16"


def run_device(d, x0, A, B, C, D, B2, C2, D12, D21, *, trace=False,
               nsteps=NSTEPS, ncores=NCORES, trace_kwargs=None, mm_dt=None):
    mm_dt = mm_dt or MM_DT
    bass_dt = {"f32": F32, "f32r": mybir.dt.float32r,
               "f16": mybir.dt.float16}[mm_dt]
    np_dt = {"f32": np.float32, "f32r": np.float32, "f16": np.float16}[mm_dt]

    d = np.asarray(d, np.float32)
    x0 = np.asarray(x0, np.float32)
    bfull = d.shape[0]
    bc = bfull // ncores

    weights = {k: v.astype(np_dt)
               for k, v in _host_weights(A, B, C, D, B2, C2, D12, D21).items()}
    nc = _build_bass(nsteps, bc, mm_dt=bass_dt)

    in_maps = []
    for c in range(ncores):
        ds = d[c * bc:(c + 1) * bc, :nsteps, :, 0]        # [bc, nsteps, ND]
        x0s = x0[c * bc:(c + 1) * bc, :, 0]               # [bc, NX]
        in_maps.append({
            "d_dup": _make_d_dup(ds, nsteps, bc).astype(np_dt),
            "x0": np.ascontiguousarray(x0s.T, np_dt),     # [64, bc]
            **weights,
        })

    res = run_bass_kernel_spmd(nc, in_maps, core_ids=list(range(ncores)),
                               trace=trace, **(trace_kwargs or {}))

    e_hat = np.empty((bfull, nsteps, NE, 1), np.float32)
    w = np.empty((bfull, nsteps, NW, 1), np.float32)
    x = np.empty((bfull, nsteps + 1, NX, 1), np.float32)
    for c in range(ncores):
        sx = res.results[c]["sx_out"].astype(np.float32)  # [128, nsteps+1, bc]
        ec = res.results[c]["e_out"]                      # [16, nsteps, bc]
        sl = slice(c * bc, (c + 1) * bc)
        w[sl, :, :, 0] = np.transpose(sx[0:64, 0:nsteps, :], (2, 1, 0))
        x[sl, 1:, :, 0] = np.transpose(sx[64:128, 1:nsteps + 1, :], (2, 1, 0))
        e_hat[sl, :, :, 0] = np.transpose(ec, (2, 1, 0))
    x[:, 0, :, 0] = x0[:, :, 0]
    return (e_hat, x, w), res


def kernel(d, x0, A, B, C, D, B2, C2, D12, D21):
    (e_hat, x, w), _ = run_device(d, x0, A, B, C, D, B2, C2, D12, D21)
    d = np.asarray(d, np.float32)
    return (e_hat, (x, w), d)


# revision 12
# speedup vs baseline: 2.0089x; 1.2907x over previous
"""Trainium2 Bass kernel for the Lure-system rollout.

Math (per batch column, all fp32):
    w_t = tanh(C2 x_t + D21 d_t)
    e_t = C x_t + D d_t + D12 w_t
    x_{t+1} = A x_t + B d_t + B2 w_t

Device formulation (per core, batch shard of 32 columns):
    state tile s_t = [w_t ; x_t]  (128 partitions x 32 batch)
    psum tile P_{t+1} = [z_{t+1} ; x_{t+1}]:
        P_{t+1}  = Wd^T @ [d_t ; d_{t+1}]          (start)
        P_{t+1} += Ws^T @ s_t                      (stop)
      where Ws packs [G0=C2B2, C2A | B2, A] and Wd packs [C2B, D21 | B, 0].
    w_{t+1} = tanh(P_{t+1}[0:64])  (ACT, PSUM->SBUF)
    x_{t+1} = copy P_{t+1}[64:128] (DVE, PSUM->SBUF)
    e computed in bulk every 16 steps from the [w;x] history (off critical path).

The serial critical path is exactly: tanh -> Ws-matmul -> tanh (2 sem hops/step).
"""

import numpy as np

import concourse.bacc as bacc
import concourse.bass as bass
import concourse.mybir as mybir
import concourse.tile as tile
from concourse.bass_utils import run_bass_kernel_spmd

NX, ND, NE, NW = 64, 16, 16, 64
BFULL, NSTEPS = 256, 1024
NCORES = 8
BC = BFULL // NCORES  # batch per core

F32 = mybir.dt.float32


def _host_weights(A, B, C, D, B2, C2, D12, D21):
    A, B, C, D = (np.asarray(m, np.float64) for m in (A, B, C, D))
    B2, C2, D12, D21 = (np.asarray(m, np.float64) for m in (B2, C2, D12, D21))
    G0 = C2 @ B2          # [64, 64] w -> z
    C2A = C2 @ A          # [64, 64] x -> z
    C2B = C2 @ B          # [64, 16] d_t -> z

    Ws = np.zeros((128, 128), np.float64)
    Ws[0:64, 0:64] = G0.T
    Ws[64:128, 0:64] = C2A.T
    Ws[0:64, 64:128] = B2.T
    Ws[64:128, 64:128] = A.T

    Wd = np.zeros((32, 128), np.float64)
    Wd[0:16, 0:64] = C2B.T
    Wd[16:32, 0:64] = D21.T
    Wd[0:16, 64:128] = B.T

    Wdm1 = np.zeros((32, 64), np.float64)
    Wdm1[0:16, :] = D21.T

    W0 = C2.T  # [64, 64], used at partition base 64 for z_0 += C2 x_0

    We = np.zeros((128, 16), np.float64)
    We[0:64, :] = D12.T
    We[64:128, :] = C.T

    WeD = D.T  # [16, 16]

    f32 = np.float32
    return {
        "Ws": np.ascontiguousarray(Ws, f32),
        "Wd": np.ascontiguousarray(Wd, f32),
        "Wdm1": np.ascontiguousarray(Wdm1, f32),
        "W0": np.ascontiguousarray(W0, f32),
        "We": np.ascontiguousarray(We, f32),
        "WeD": np.ascontiguousarray(WeD, f32),
    }


def _build_bass(nsteps, bc, ch=256, dma_chunk=64, e_chunk=16, mm_dt=F32):
    """Emit the Bass/Tile program. Same NEFF on all cores (pure data parallel).

    mm_dt controls the dtype of the recurrence matmuls (weights, state
    history, d staging). PSUM stays fp32 throughout.
    """
    nc = bacc.Bacc("TRN2", target_bir_lowering=False, debug=False)

    DT = mm_dt
    d_dup = nc.dram_tensor("d_dup", [32, nsteps, bc], DT, kind="ExternalInput").ap()
    x0 = nc.dram_tensor("x0", [64, bc], DT, kind="ExternalInput").ap()
    Ws = nc.dram_tensor("Ws", [128, 128], DT, kind="ExternalInput").ap()
    Wd = nc.dram_tensor("Wd", [32, 128], DT, kind="ExternalInput").ap()
    Wdm1 = nc.dram_tensor("Wdm1", [32, 64], DT, kind="ExternalInput").ap()
    W0 = nc.dram_tensor("W0", [64, 64], DT, kind="ExternalInput").ap()
    We = nc.dram_tensor("We", [128, 16], DT, kind="ExternalInput").ap()
    WeD = nc.dram_tensor("WeD", [16, 16], DT, kind="ExternalInput").ap()

    sx_out = nc.dram_tensor("sx_out", [128, nsteps + 1, bc], DT,
                            kind="ExternalOutput").ap()
    e_out = nc.dram_tensor("e_out", [16, nsteps, bc], F32,
                           kind="ExternalOutput").ap()

    assert ch % dma_chunk == 0 and ch % e_chunk == 0
    assert nsteps % dma_chunk == 0

    tanh = mybir.ActivationFunctionType.Tanh

    with tile.TileContext(nc) as tc:
        with (
            tc.tile_pool(name="const", bufs=1) as const,
            tc.tile_pool(name="hist", bufs=1) as hist,
            tc.tile_pool(name="pp", bufs=4, space="PSUM") as pp,
            tc.tile_pool(name="pe", bufs=2, space="PSUM") as pe,
        ):
            ws_sb = const.tile([128, 128], DT)
            wd_sb = const.tile([32, 128], DT)
            wdm1_sb = const.tile([32, 64], DT)
            w0_sb = const.tile([128, 64], DT)   # C2^T lives at partitions 64:128
            we_sb = const.tile([128, 16], DT)
            wed_sb = const.tile([16, 16], DT)
            nc.sync.dma_start(out=ws_sb, in_=Ws)
            nc.sync.dma_start(out=wd_sb, in_=Wd)
            nc.sync.dma_start(out=wdm1_sb, in_=Wdm1)
            nc.sync.dma_start(out=w0_sb[64:128, :], in_=W0)
            nc.sync.dma_start(out=we_sb, in_=We)
            nc.sync.dma_start(out=wed_sb, in_=WeD)

            s_hist = hist.tile([128, ch, bc], DT)   # [w ; x] per slot
            d_sb = hist.tile([32, ch, bc], DT)      # [d_t ; d_{t+1}] per slot
            e_stage = hist.tile([16, ch, bc], F32)

            nc.sync.dma_start(out=s_hist[64:128, 0, :], in_=x0)

            # d chunks 0 and 1 up front
            pre = min(2 * dma_chunk, nsteps)
            nc.sync.dma_start(out=d_sb[:, 0:pre, :], in_=d_dup[:, 0:pre, :])

            # P_0 = [z_0] = D21 d_0 + C2 x_0
            p_cur = pp.tile([64, bc], F32, tag="p0", bufs=1)
            nc.tensor.matmul(p_cur, wdm1_sb, d_sb[:, 0, :], start=True, stop=False)
            nc.tensor.matmul(p_cur, w0_sb[64:128, :], s_hist[64:128, 0, :],
                             start=False, stop=True)

            for t in range(nsteps):
                slot = t % ch

                # prefetch d chunk t//dma_chunk + 2
                if t % dma_chunk == 0:
                    lo = (t // dma_chunk + 2) * dma_chunk
                    if lo < nsteps:
                        hi = min(lo + dma_chunk, nsteps)
                        sl = lo % ch
                        nc.sync.dma_start(out=d_sb[:, sl:sl + (hi - lo), :],
                                          in_=d_dup[:, lo:hi, :])

                # w_t = tanh(z_t)
                nc.scalar.activation(out=s_hist[0:64, slot, :],
                                     in_=p_cur[0:64, :], func=tanh)

                # P_{t+1} = Wd^T [d_t; d_{t+1}] + Ws^T [w_t; x_t]
                # Split Ws by K: x-rows (ready early, off critical path) then
                # w-rows (the only matmul gated on tanh_t; its single wait
                # stays on the MATMUL so the LDWEIGHTS prefetches during tanh).
                p_next = pp.tile([128, bc], F32, tag="p")
                nc.tensor.matmul(p_next, wd_sb, d_sb[:, slot, :],
                                 start=True, stop=False)
                nc.tensor.matmul(p_next, ws_sb[64:128, :],
                                 s_hist[64:128, slot, :],
                                 start=False, stop=False)
                nc.tensor.matmul(p_next, ws_sb[0:64, :], s_hist[0:64, slot, :],
                                 start=False, stop=True)
                nc.vector.tensor_copy(s_hist[64:128, (t + 1) % ch, :],
                                      p_next[64:128, :])
                p_cur = p_next

                # e for steps [t-e_chunk+1, t]
                if t % e_chunk == e_chunk - 1:
                    sl0 = (t - e_chunk + 1) % ch
                    ep = pe.tile([16, e_chunk * bc], F32, tag="e")
                    nc.tensor.matmul(ep, we_sb, s_hist[:, sl0:sl0 + e_chunk, :],
                                     start=True, stop=False)
                    nc.tensor.matmul(ep, wed_sb, d_sb[0:16, sl0:sl0 + e_chunk, :],
                                     start=False, stop=True)
                    nc.vector.tensor_copy(e_stage[:, sl0:sl0 + e_chunk, :], ep)

                # stores
                if t % dma_chunk == dma_chunk - 1:
                    lo = t - dma_chunk + 1
                    sl = lo % ch
                    nc.sync.dma_start(out=sx_out[:, lo:t + 1, :],
                                      in_=s_hist[:, sl:sl + dma_chunk, :])
                e_flush = min(2 * dma_chunk, nsteps)
                if t % e_flush == e_flush - 1:
                    lo = t - e_flush + 1
                    sl = lo % ch
                    nc.sync.dma_start(out=e_out[:, lo:t + 1, :],
                                      in_=e_stage[:, sl:sl + e_flush, :])

            # final x_{nsteps} (written by step nsteps-1's copy)
            sl = nsteps % ch
            nc.sync.dma_start(out=sx_out[64:128, nsteps:nsteps + 1, :],
                              in_=s_hist[64:128, sl:sl + 1, :])

    nc.compile()
    return nc


def _make_d_dup(d_shard, nsteps, bc):
    # d_shard: [bc, nsteps, ND] -> [32, nsteps, bc] with rows 0:16 = d_t,
    # rows 16:32 = d_{t+1} (zeros at t = nsteps-1).
    dT = np.ascontiguousarray(np.transpose(d_shard, (2, 1, 0)), np.float32)
    d_dup = np.zeros((32, nsteps, bc), np.float32)
    d_dup[0:16] = dT
    d_dup[16:32, :-1] = dT[:, 1:]
    return d_dup


MM_DT = "f16"  # "f32" | "f32r" | "f16"


def run_device(d, x0, A, B, C, D, B2, C2, D12, D21, *, trace=False,
               nsteps=NSTEPS, ncores=NCORES, trace_kwargs=None, mm_dt=None):
    mm_dt = mm_dt or MM_DT
    bass_dt = {"f32": F32, "f32r": mybir.dt.float32r,
               "f16": mybir.dt.float16}[mm_dt]
    np_dt = {"f32": np.float32, "f32r": np.float32, "f16": np.float16}[mm_dt]

    d = np.asarray(d, np.float32)
    x0 = np.asarray(x0, np.float32)
    bfull = d.shape[0]
    bc = bfull // ncores

    weights = {k: v.astype(np_dt)
               for k, v in _host_weights(A, B, C, D, B2, C2, D12, D21).items()}
    nc = _build_bass(nsteps, bc, mm_dt=bass_dt)

    in_maps = []
    for c in range(ncores):
        ds = d[c * bc:(c + 1) * bc, :nsteps, :, 0]        # [bc, nsteps, ND]
        x0s = x0[c * bc:(c + 1) * bc, :, 0]               # [bc, NX]
        in_maps.append({
            "d_dup": _make_d_dup(ds, nsteps, bc).astype(np_dt),
            "x0": np.ascontiguousarray(x0s.T, np_dt),     # [64, bc]
            **weights,
        })

    res = run_bass_kernel_spmd(nc, in_maps, core_ids=list(range(ncores)),
                               trace=trace, **(trace_kwargs or {}))

    e_hat = np.empty((bfull, nsteps, NE, 1), np.float32)
    w = np.empty((bfull, nsteps, NW, 1), np.float32)
    x = np.empty((bfull, nsteps + 1, NX, 1), np.float32)
    for c in range(ncores):
        sx = res.results[c]["sx_out"].astype(np.float32)  # [128, nsteps+1, bc]
        ec = res.results[c]["e_out"]                      # [16, nsteps, bc]
        sl = slice(c * bc, (c + 1) * bc)
        w[sl, :, :, 0] = np.transpose(sx[0:64, 0:nsteps, :], (2, 1, 0))
        x[sl, 1:, :, 0] = np.transpose(sx[64:128, 1:nsteps + 1, :], (2, 1, 0))
        e_hat[sl, :, :, 0] = np.transpose(ec, (2, 1, 0))
    x[:, 0, :, 0] = x0[:, :, 0]
    return (e_hat, x, w), res


def kernel(d, x0, A, B, C, D, B2, C2, D12, D21):
    (e_hat, x, w), _ = run_device(d, x0, A, B, C, D, B2, C2, D12, D21)
    d = np.asarray(d, np.float32)
    return (e_hat, (x, w), d)
